# revision 35
# baseline (speedup 1.0000x reference)
"""DeepSeek-MoE layer on 8 TRN2 NeuronCores.

Strategy (intermediate-dim parallel, host-side dispatch):
  - Router (x @ gate_w.T, softmax, top-2) computed on host — it *is* the
    sharding decision (~0.02% of total FLOPs).
  - Every core computes a 384-wide I-slice (1/8 of I=3072) of ALL 8
    routed experts over their routed tokens plus BOTH shared experts
    over all 2048 tokens.  Work per core is exactly 1/8 of the total
    MACs regardless of routing skew — no max-expert-load capacity
    penalty (expert-parallel pays max_e n_e on every core).
  - Cores emit partial down-projections (their I-slice's contribution,
    bf16); the host sums the 8 partials in fp32 and applies the combine
    weights / scatter-add.
  - All matmuls bf16 (fp32 PSUM accumulation).

Device kernel layout (driven by trace analysis):
  - Each dma_start costs ~0.65us of serialized HWDGE sequencer time, so
    transfers are merged: ONE weight DMA per expert (1.77 MB), ONE x
    DMA per col-tile segment, ONE merged output store per segment
    (expert 0 / segment 0 are split finer to shorten the critical path
    to the first matmul).
  - PSUM: 2 gate/up banks + 6 down banks = 8.  The six down-proj
    accumulations of k-slice j are emitted right after gate/up of
    slice j+1, so PE bubbles from the silu/mul round trip are filled
    and down banks accumulate across all h tiles with no half-drain.
  - ~30 dummy N=256 matmuls on a memset tile run during the initial
    DMA wait so the PE_HAM clock gate is at 8/8 before real work.
  - Shared experts' down projections accumulate into the same PSUM
    banks (6-step) so the S0+S1 sum leaves the chip as one tensor.
"""
import os
import sys
import types

import numpy as np
import ml_dtypes

import concourse.bass as bass
import concourse.tile as tile
import concourse.mybir as mybir
from concourse import bacc
from concourse.bass_utils import run_bass_kernel_spmd

# ---- problem constants (DeepSeekMoE: B=2,S=1024,H=768,I=3072,E=8,NS=2,k=2) --
H = 768          # hidden
I = 3072         # intermediate
E = 8            # routed experts
NS = 2           # shared experts
TOP_K = 2
N_CORES = 8
KH = H // 128        # 6 k-tiles over H
ISL = I // N_CORES   # 384: per-core I-slice
MI = ISL // 128      # 3 mi-tiles per core per expert
T_TOT = 2048
RCOLS = T_TOT * TOP_K      # 4096 routed columns
SCOLS = T_TOT              # 2048 shared columns
NCOL = RCOLS + SCOLS       # 6144
CHUNK = KH * ISL           # 2304 elements per partition per weight matrix
WROW = 3 * CHUNK           # 6912: gate ++ up ++ down image per expert
N_EXP = E + NS             # 10 weight sets
WARM_MMS = 30

BF16 = mybir.dt.bfloat16
F32 = mybir.dt.float32
_bf = ml_dtypes.bfloat16


def _install_ntff_hook():
    """Provide antenv.axon_hooks (missing on this image) so trace=True works."""
    if "antenv.axon_hooks" in sys.modules:
        return
    try:
        from trn_agent_boot.trn_boot import _ntff_profile_via_ctypes
        hook = _ntff_profile_via_ctypes("/opt/axon/libaxon_pjrt.so")
    except Exception:
        hook = None
    mod = types.ModuleType("antenv.axon_hooks")
    mod.get_axon_ntff_profile_hook = lambda: hook
    sys.modules["antenv.axon_hooks"] = mod


def _col_tiles(n):
    """Split n columns into tiles <= 512."""
    if n == 0:
        return []
    nt = -(-n // 512)
    base = n // nt
    return [base + (1 if i < n % nt else 0) for i in range(nt)]


def _segments(plan):
    """Segment list: (kind, expert, col_offset, n_cols).  The shared range
    ends with a small 128-col segment so the final drain tail is short."""
    segs = []
    off = 0
    for e in range(E):
        for n in _col_tiles(plan[e]):
            segs.append(("r", e, off, n))
            off += n
    assert off == RCOLS
    for n in (512, 512, 512, 384, 128):
        segs.append(("s", 0, off, n))
        off += n
    assert off == NCOL
    return segs


def build_nc(plan):
    """Build the SPMD program.  plan: per-routed-expert token counts."""
    nc = bacc.Bacc(None, target_bir_lowering=False)
    X = nc.dram_tensor("x", [128, KH * NCOL], BF16, kind="ExternalInput")
    WCH = nc.dram_tensor("wch", [N_EXP, 128, WROW], BF16,
                         kind="ExternalInput")
    Y = nc.dram_tensor("y", [128, KH * NCOL], BF16, kind="ExternalOutput")

    segs = _segments(plan)
    nseg = len(segs)
    soff = [KH * s[2] for s in segs]  # segment-major element offsets

    with tile.TileContext(nc) as tc:
        with tc.tile_pool(name="wpool", bufs=4) as wpool, \
             tc.tile_pool(name="xpool", bufs=5) as xpool, \
             tc.tile_pool(name="hpool", bufs=10) as hpool, \
             tc.tile_pool(name="sgpool", bufs=3) as sgpool, \
             tc.tile_pool(name="ystage", bufs=6) as ystage, \
             tc.tile_pool(name="warmp", bufs=1) as warmp, \
             tc.tile_pool(name="gupool", bufs=2, space="PSUM") as gupool, \
             tc.tile_pool(name="ypool", bufs=6, space="PSUM") as ypool:

            # ---- HAM pre-warm: dummy matmuls on a memset tile ----------
            wsb = warmp.tile([128, 400], BF16, tag="warm", name="warmsrc")
            nc.vector.memset(wsb, 0.0)
            warm = gupool.tile([128, 512], F32, tag="gu", name="warmps")
            for _ in range(WARM_MMS):
                nc.tensor.matmul(warm[:, :256], wsb[:, :128], wsb[:, 128:384],
                                 start=True, stop=True)

            w_tiles = {}     # expert id (0..7 routed, 8/9 shared) -> tile
            x_tiles = {}     # segment index -> x tile

            def load_w(eid, split=False):
                if eid in w_tiles:
                    return
                wt = wpool.tile([128, WROW], BF16, tag="w")
                if split:
                    # finer pieces so the first matmuls wait on ~0.3MB
                    cuts = [0, CHUNK // 2, CHUNK, CHUNK + CHUNK // 2,
                            2 * CHUNK, WROW]
                else:
                    cuts = [0, WROW]
                for lo, hi in zip(cuts[:-1], cuts[1:]):
                    nc.sync.dma_start(out=wt[:, lo:hi],
                                      in_=WCH[eid, :, lo:hi])
                w_tiles[eid] = wt

            def issue_seg(si):
                kind, e, oc, n = segs[si]
                xt = xpool.tile([128, KH * 512], BF16, tag="x")
                so = soff[si]
                # x on the scalar ring, weights on sync: the two head
                # streams transfer concurrently, and stores (also scalar)
                # are issued lazily so they self-pace behind x loads
                nc.scalar.dma_start(out=xt[:, :KH * n],
                                    in_=X[:, so:so + KH * n])
                x_tiles[si] = xt
                if kind == "r":
                    load_w(e, split=(si == 0))
                else:
                    load_w(E)
                    load_w(E + 1)

            seg_ctr = [0]

            def compute_seg(si):
                kind, e, oc, n = segs[si]
                xt = x_tiles.pop(si)
                eids = [e] if kind == "r" else [E, E + 1]
                nacc = len(eids) * MI
                ys = [ypool.tile([128, 512], F32, tag="y", name=f"y{t}")
                      for t in range(KH)]
                hs = {}

                def emit_down(gk):
                    eid_, j = eids[gk // MI], gk % MI
                    td = w_tiles[eid_]
                    for t in range(KH):
                        nc.tensor.matmul(
                            ys[t][:, :n],
                            td[:, 2 * CHUNK + j * H + t * 128:
                               2 * CHUNK + j * H + (t + 1) * 128],
                            hs[gk][:, :n],
                            start=(gk == 0), stop=(gk == nacc - 1))

                for ei, eid in enumerate(eids):
                    wt = w_tiles[eid]
                    for mi in range(MI):
                        gk = ei * MI + mi
                        g = gupool.tile([128, 512], F32, tag="gu")
                        for k in range(KH):
                            o = k * ISL + mi * 128
                            nc.tensor.matmul(
                                g[:, :n], wt[:, o:o + 128],
                                xt[:, k * n:k * n + n],
                                start=(k == 0), stop=(k == KH - 1))
                        u = gupool.tile([128, 512], F32, tag="gu")
                        for k in range(KH):
                            o = CHUNK + k * ISL + mi * 128
                            nc.tensor.matmul(
                                u[:, :n], wt[:, o:o + 128],
                                xt[:, k * n:k * n + n],
                                start=(k == 0), stop=(k == KH - 1))
                        sg = sgpool.tile([128, 512], F32, tag="sg")
                        nc.scalar.activation(sg[:, :n], g[:, :n],
                                             mybir.ActivationFunctionType.Silu)
                        h = hpool.tile([128, 512], BF16, tag="h")
                        nc.vector.tensor_mul(h[:, :n], sg[:, :n], u[:, :n])
                        hs[gk] = h
                        # down-proj lags two h tiles behind so its last
                        # groups never wait on the final silu/mul chain
                        if gk > 1:
                            emit_down(gk - 2)
                emit_down(nacc - 2)
                emit_down(nacc - 1)

                st = ystage.tile([128, KH * 512], BF16, tag="yst")
                for t in range(KH):
                    # alternate engines so drains don't serialize; the last
                    # two (small) segments drain on DVE only so no scalar
                    # COPY can delay the final silu->mul->down chain
                    if t % 2 == 0 or si >= nseg - 2:
                        nc.vector.tensor_copy(st[:, t * n:(t + 1) * n],
                                              ys[t][:, :n])
                    else:
                        nc.scalar.copy(st[:, t * n:(t + 1) * n],
                                       ys[t][:, :n])
                so = soff[si]
                nc.scalar.dma_start(out=Y[:, so:so + KH * n],
                                    in_=st[:, :KH * n])
                seg_ctr[0] += 1

            PF = 3  # segment prefetch depth
            for si in range(min(PF, nseg)):
                issue_seg(si)
            # preload the ACT COPY/SILU tables off the critical path (the
            # first use would stall the scalar queue ~1.3us each); placed
            # after the head x DMAs so they don't delay x0
            nc.scalar.activation(wsb[:, 384:392], wsb[:, 392:400],
                                 mybir.ActivationFunctionType.Silu)
            nc.scalar.copy(wsb[:, 384:392], wsb[:, 392:400])
            for si in range(nseg):
                if si + PF < nseg:
                    issue_seg(si + PF)
                compute_seg(si)
    nc.finalize()
    return nc


def _chunk_gu(wT, c):
    """[H, I] lhsT-layout weight -> this core's [128, 2304] gate/up image.
    chunk[p, k*384 + m] = wT[k*128 + p, c*384 + m]"""
    a = wT[:, c * ISL:(c + 1) * ISL].reshape(KH, 128, ISL)
    return np.ascontiguousarray(a.transpose(1, 0, 2)).reshape(128, CHUNK)


def _chunk_d(dT, c):
    """[I, H] lhsT-layout down weight -> this core's [128, 2304] image.
    chunk[p, j*768 + i] = dT[c*384 + j*128 + p, i]"""
    a = dT[c * ISL:(c + 1) * ISL, :].reshape(MI, 128, H)
    return np.ascontiguousarray(a.transpose(1, 0, 2)).reshape(128, CHUNK)


_NC_CACHE = {}


def kernel(hidden_states, gate_w, shared_gate, shared_up, shared_down,
           routed_gate, routed_up, routed_down):
    B, S, _ = hidden_states.shape
    T = B * S
    x = np.asarray(hidden_states, np.float32).reshape(T, H)

    # ---- host router (mirrors reference math; fp64 softmax for stability) --
    logits = x @ np.asarray(gate_w, np.float32).T                    # [T, E]
    lg = logits.astype(np.float64)
    sc = np.exp(lg - lg.max(1, keepdims=True))
    sc /= sc.sum(1, keepdims=True)
    topk_idx = np.argsort(-sc, axis=1, kind="stable")[:, :TOP_K]     # [T, k]
    topk_w = np.take_along_axis(sc, topk_idx, axis=1)
    topk_w = topk_w / (topk_w.sum(1, keepdims=True) + 1e-8)          # [T, k]

    tok_lists = []
    tok_weights = []
    for e in range(E):
        sel = (topk_idx == e)
        toks = np.where(sel.any(1))[0]
        w = (topk_w * sel)[toks].sum(1).astype(np.float32)
        tok_lists.append(toks)
        tok_weights.append(w)
    plan = tuple(len(t) for t in tok_lists)
    assert sum(plan) == RCOLS
    tok_concat = np.concatenate(tok_lists)                 # [4096]
    w_concat = np.concatenate(tok_weights)                 # [4096]

    # ---- x image [128, KH*NCOL], segment-major so each segment's load is
    # one fully contiguous DMA: seg block [p, k*n + j] = x[col oc+j, k*128+p]
    x_bf = x.astype(_bf)
    xT = np.empty((H, NCOL), _bf)
    xT[:, :RCOLS] = x_bf[tok_concat].T
    xT[:, RCOLS:] = x_bf.T
    xk = xT.reshape(KH, 128, NCOL)
    segs = _segments(plan)
    ximg = np.empty((128, KH * NCOL), _bf)
    for _, _, oc, n in segs:
        blk = xk[:, :, oc:oc + n].transpose(1, 0, 2).reshape(128, KH * n)
        ximg[:, KH * oc:KH * (oc + n)] = blk

    # ---- per-core weight images ------------------------------------------
    gT = [np.ascontiguousarray(np.asarray(routed_gate[e], np.float32).T)
          .astype(_bf) for e in range(E)]
    uT = [np.ascontiguousarray(np.asarray(routed_up[e], np.float32).T)
          .astype(_bf) for e in range(E)]
    dT = [np.ascontiguousarray(np.asarray(routed_down[e], np.float32).T)
          .astype(_bf) for e in range(E)]
    for s in range(NS):
        gT.append(np.ascontiguousarray(
            np.asarray(shared_gate[s], np.float32).T).astype(_bf))
        uT.append(np.ascontiguousarray(
            np.asarray(shared_up[s], np.float32).T).astype(_bf))
        dT.append(np.ascontiguousarray(
            np.asarray(shared_down[s], np.float32).T).astype(_bf))

    in_maps = []
    for c in range(N_CORES):
        wch = np.empty((N_EXP, 128, WROW), _bf)
        for e in range(N_EXP):
            wch[e, :, :CHUNK] = _chunk_gu(gT[e], c)
            wch[e, :, CHUNK:2 * CHUNK] = _chunk_gu(uT[e], c)
            wch[e, :, 2 * CHUNK:] = _chunk_d(dT[e], c)
        in_maps.append({"x": ximg, "wch": wch})

    # ---- build + run on 8 cores -----------------------------------------
    if plan not in _NC_CACHE:
        _NC_CACHE.clear()
        _NC_CACHE[plan] = build_nc(plan)
    nc = _NC_CACHE[plan]

    trace = bool(int(os.environ.get("MOE_TRACE", "0")))
    kw = {}
    if trace:
        _install_ntff_hook()
        kw = dict(trace=True, trace_cores=list(range(N_CORES)))
    res = run_bass_kernel_spmd(nc, in_maps, core_ids=list(range(N_CORES)), **kw)
    if trace:
        print(f"HW exec time: {res.exec_time_ns} ns")

    # ---- host combine: sum I-slice partials, weight, scatter -------------
    acc = np.zeros((128, KH * NCOL), np.float32)
    for c in range(N_CORES):
        acc += res.results[c]["y"].astype(np.float32)
    accT = np.empty((H, NCOL), np.float32)                 # [768, 6144]
    for _, _, oc, n in segs:
        blk = acc[:, KH * oc:KH * (oc + n)].reshape(128, KH, n)
        accT[:, oc:oc + n] = blk.transpose(1, 0, 2).reshape(H, n)
    out = np.zeros((T, H), np.float32)
    np.add.at(out, tok_concat, accT[:, :RCOLS].T * w_concat[:, None])
    out += accT[:, RCOLS:].T / NS
    return out.reshape(B, S, H)


# revision 39
# speedup vs baseline: 1.0222x; 1.0222x over previous
"""DeepSeek-MoE layer on 8 TRN2 NeuronCores.

Strategy (intermediate-dim parallel, host-side dispatch):
  - Router (x @ gate_w.T, softmax, top-2) computed on host — it *is* the
    sharding decision (~0.02% of total FLOPs).
  - Every core computes a 384-wide I-slice (1/8 of I=3072) of ALL 8
    routed experts over their routed tokens plus BOTH shared experts
    over all 2048 tokens.  Work per core is exactly 1/8 of the total
    MACs regardless of routing skew — no max-expert-load capacity
    penalty (expert-parallel pays max_e n_e on every core).
  - Cores emit partial down-projections (their I-slice's contribution,
    bf16); the host sums the 8 partials in fp32 and applies the combine
    weights / scatter-add.
  - All matmuls bf16 (fp32 PSUM accumulation).

Device kernel layout (driven by trace analysis):
  - Each dma_start costs ~0.65us of serialized HWDGE sequencer time, so
    transfers are merged: ONE weight DMA per expert (1.77 MB), ONE x
    DMA per col-tile segment, ONE merged output store per segment
    (expert 0 / segment 0 are split finer to shorten the critical path
    to the first matmul).
  - PSUM: 2 gate/up banks + 6 down banks = 8.  The six down-proj
    accumulations of k-slice j are emitted right after gate/up of
    slice j+1, so PE bubbles from the silu/mul round trip are filled
    and down banks accumulate across all h tiles with no half-drain.
  - 32 dummy N=256 matmuls on a memset tile run during the initial
    DMA wait so the PE_HAM clock gate is at 8/8 before real work
    (and so the idle-window monitor never re-throttles the PE clock).
  - Shared experts' down projections accumulate into the same PSUM
    banks (6-step) so the S0+S1 sum leaves the chip as one tensor.
"""
import os
import sys
import types

import numpy as np
import ml_dtypes

import concourse.bass as bass
import concourse.tile as tile
import concourse.mybir as mybir
from concourse import bacc
from concourse.bass_utils import run_bass_kernel_spmd

# ---- problem constants (DeepSeekMoE: B=2,S=1024,H=768,I=3072,E=8,NS=2,k=2) --
H = 768          # hidden
I = 3072         # intermediate
E = 8            # routed experts
NS = 2           # shared experts
TOP_K = 2
N_CORES = 8
KH = H // 128        # 6 k-tiles over H
ISL = I // N_CORES   # 384: per-core I-slice
MI = ISL // 128      # 3 mi-tiles per core per expert
T_TOT = 2048
RCOLS = T_TOT * TOP_K      # 4096 routed columns
SCOLS = T_TOT              # 2048 shared columns
NCOL = RCOLS + SCOLS       # 6144
CHUNK = KH * ISL           # 2304 elements per partition per weight matrix
WROW = 3 * CHUNK           # 6912: gate ++ up ++ down image per expert
N_EXP = E + NS             # 10 weight sets
WARM_MMS = 32

BF16 = mybir.dt.bfloat16
F32 = mybir.dt.float32
_bf = ml_dtypes.bfloat16


def _install_ntff_hook():
    """Provide antenv.axon_hooks (missing on this image) so trace=True works."""
    if "antenv.axon_hooks" in sys.modules:
        return
    try:
        from trn_agent_boot.trn_boot import _ntff_profile_via_ctypes
        hook = _ntff_profile_via_ctypes("/opt/axon/libaxon_pjrt.so")
    except Exception:
        hook = None
    mod = types.ModuleType("antenv.axon_hooks")
    mod.get_axon_ntff_profile_hook = lambda: hook
    sys.modules["antenv.axon_hooks"] = mod


def _col_tiles(n):
    """Split n columns into tiles <= 512."""
    if n == 0:
        return []
    nt = -(-n // 512)
    base = n // nt
    return [base + (1 if i < n % nt else 0) for i in range(nt)]


def _segments(plan):
    """Segment list: (kind, expert, col_offset, n_cols).  The shared range
    ends with a small 128-col segment so the final drain tail is short."""
    segs = []
    off = 0
    for e in range(E):
        for n in _col_tiles(plan[e]):
            segs.append(("r", e, off, n))
            off += n
    assert off == RCOLS
    for n in (512, 512, 512, 384, 128):
        segs.append(("s", 0, off, n))
        off += n
    assert off == NCOL
    return segs


def build_nc(plan):
    """Build the SPMD program.  plan: per-routed-expert token counts."""
    nc = bacc.Bacc(None, target_bir_lowering=False)
    X = nc.dram_tensor("x", [128, KH * NCOL], BF16, kind="ExternalInput")
    WCH = nc.dram_tensor("wch", [N_EXP, 128, WROW], BF16,
                         kind="ExternalInput")
    Y = nc.dram_tensor("y", [128, KH * NCOL], BF16, kind="ExternalOutput")

    segs = _segments(plan)
    nseg = len(segs)
    soff = [KH * s[2] for s in segs]  # segment-major element offsets

    with tile.TileContext(nc) as tc:
        with tc.tile_pool(name="wpool", bufs=4) as wpool, \
             tc.tile_pool(name="xpool", bufs=5) as xpool, \
             tc.tile_pool(name="hpool", bufs=10) as hpool, \
             tc.tile_pool(name="sgpool", bufs=3) as sgpool, \
             tc.tile_pool(name="ystage", bufs=6) as ystage, \
             tc.tile_pool(name="warmp", bufs=1) as warmp, \
             tc.tile_pool(name="gupool", bufs=2, space="PSUM") as gupool, \
             tc.tile_pool(name="ypool", bufs=6, space="PSUM") as ypool:

            # ---- HAM pre-warm: dummy matmuls on a memset tile ----------
            wsb = warmp.tile([128, 384], BF16, tag="warm", name="warmsrc")
            nc.vector.memset(wsb, 0.0)
            warm = gupool.tile([128, 512], F32, tag="gu", name="warmps")
            for _ in range(WARM_MMS):
                nc.tensor.matmul(warm[:, :256], wsb[:, :128], wsb[:, 128:384],
                                 start=True, stop=True)
            # preload the ACT COPY activation table (sel=1) off the critical
            # path — otherwise the first PSUM drain stalls scalar ~1.3us
            nc.scalar.copy(wsb[:, :8], wsb[:, 8:16])

            w_tiles = {}     # expert id (0..7 routed, 8/9 shared) -> tile
            x_tiles = {}     # segment index -> x tile

            def load_w(eid, split=False):
                if eid in w_tiles:
                    return
                wt = wpool.tile([128, WROW], BF16, tag="w")
                if split:
                    # finer pieces so the first matmuls wait on ~0.1MB
                    cuts = [0, ISL, CHUNK, 2 * CHUNK, 3 * CHUNK]
                else:
                    cuts = [0, WROW]
                for lo, hi in zip(cuts[:-1], cuts[1:]):
                    nc.sync.dma_start(out=wt[:, lo:hi],
                                      in_=WCH[eid, :, lo:hi])
                w_tiles[eid] = wt

            def issue_seg(si):
                kind, e, oc, n = segs[si]
                xt = xpool.tile([128, KH * 512], BF16, tag="x")
                so = soff[si]
                if si == 0:
                    # startup head: x0 whole, then gate/up/down chunks in
                    # consumption order — few DMAs (issue costs ~0.65us
                    # of ring-sequencer time each), warm matmuls bridge
                    nc.sync.dma_start(out=xt[:, :KH * n],
                                      in_=X[:, so:so + KH * n])
                    wt = wpool.tile([128, WROW], BF16, tag="w")
                    for lo, hi in ((0, CHUNK), (CHUNK, 2 * CHUNK),
                                   (2 * CHUNK, WROW)):
                        nc.sync.dma_start(out=wt[:, lo:hi],
                                          in_=WCH[e, :, lo:hi])
                    w_tiles[e] = wt
                    x_tiles[si] = xt
                    return
                nc.sync.dma_start(out=xt[:, :KH * n],
                                  in_=X[:, so:so + KH * n])
                x_tiles[si] = xt
                if kind == "r":
                    load_w(e)
                else:
                    load_w(E)
                    load_w(E + 1)

            seg_ctr = [0]

            def compute_seg(si):
                kind, e, oc, n = segs[si]
                xt = x_tiles.pop(si)
                eids = [e] if kind == "r" else [E, E + 1]
                nacc = len(eids) * MI
                ys = [ypool.tile([128, 512], F32, tag="y", name=f"y{t}")
                      for t in range(KH)]
                hs = {}

                def emit_down(gk):
                    eid_, j = eids[gk // MI], gk % MI
                    td = w_tiles[eid_]
                    for t in range(KH):
                        nc.tensor.matmul(
                            ys[t][:, :n],
                            td[:, 2 * CHUNK + j * H + t * 128:
                               2 * CHUNK + j * H + (t + 1) * 128],
                            hs[gk][:, :n],
                            start=(gk == 0), stop=(gk == nacc - 1))

                for ei, eid in enumerate(eids):
                    wt = w_tiles[eid]
                    for mi in range(MI):
                        gk = ei * MI + mi
                        g = gupool.tile([128, 512], F32, tag="gu")
                        for k in range(KH):
                            o = k * ISL + mi * 128
                            nc.tensor.matmul(
                                g[:, :n], wt[:, o:o + 128],
                                xt[:, k * n:k * n + n],
                                start=(k == 0), stop=(k == KH - 1))
                        u = gupool.tile([128, 512], F32, tag="gu")
                        for k in range(KH):
                            o = CHUNK + k * ISL + mi * 128
                            nc.tensor.matmul(
                                u[:, :n], wt[:, o:o + 128],
                                xt[:, k * n:k * n + n],
                                start=(k == 0), stop=(k == KH - 1))
                        sg = sgpool.tile([128, 512], F32, tag="sg")
                        nc.scalar.activation(sg[:, :n], g[:, :n],
                                             mybir.ActivationFunctionType.Silu)
                        h = hpool.tile([128, 512], BF16, tag="h")
                        nc.vector.tensor_mul(h[:, :n], sg[:, :n], u[:, :n])
                        hs[gk] = h
                        # down-proj lags two h tiles behind so its last
                        # groups never wait on the final silu/mul chain
                        if gk > 1:
                            emit_down(gk - 2)
                emit_down(nacc - 2)
                emit_down(nacc - 1)

                st = ystage.tile([128, KH * 512], BF16, tag="yst")
                for t in range(KH):
                    # alternate DVE/ACT so drains never serialize one queue
                    if t % 2 == 0:
                        nc.vector.tensor_copy(st[:, t * n:(t + 1) * n],
                                              ys[t][:, :n])
                    else:
                        nc.scalar.copy(st[:, t * n:(t + 1) * n],
                                       ys[t][:, :n])
                so = soff[si]
                nc.scalar.dma_start(out=Y[:, so:so + KH * n],
                                    in_=st[:, :KH * n])
                seg_ctr[0] += 1

            PF = 3  # segment prefetch depth
            for si in range(min(PF, nseg)):
                issue_seg(si)
            for si in range(nseg):
                if si + PF < nseg:
                    issue_seg(si + PF)
                compute_seg(si)
    nc.finalize()
    return nc


def _chunk_gu(wT, c):
    """[H, I] lhsT-layout weight -> this core's [128, 2304] gate/up image.
    chunk[p, k*384 + m] = wT[k*128 + p, c*384 + m]"""
    a = wT[:, c * ISL:(c + 1) * ISL].reshape(KH, 128, ISL)
    return np.ascontiguousarray(a.transpose(1, 0, 2)).reshape(128, CHUNK)


def _chunk_d(dT, c):
    """[I, H] lhsT-layout down weight -> this core's [128, 2304] image.
    chunk[p, j*768 + i] = dT[c*384 + j*128 + p, i]"""
    a = dT[c * ISL:(c + 1) * ISL, :].reshape(MI, 128, H)
    return np.ascontiguousarray(a.transpose(1, 0, 2)).reshape(128, CHUNK)


_NC_CACHE = {}


def kernel(hidden_states, gate_w, shared_gate, shared_up, shared_down,
           routed_gate, routed_up, routed_down):
    B, S, _ = hidden_states.shape
    T = B * S
    x = np.asarray(hidden_states, np.float32).reshape(T, H)

    # ---- host router (mirrors reference math; fp64 softmax for stability) --
    logits = x @ np.asarray(gate_w, np.float32).T                    # [T, E]
    lg = logits.astype(np.float64)
    sc = np.exp(lg - lg.max(1, keepdims=True))
    sc /= sc.sum(1, keepdims=True)
    topk_idx = np.argsort(-sc, axis=1, kind="stable")[:, :TOP_K]     # [T, k]
    topk_w = np.take_along_axis(sc, topk_idx, axis=1)
    topk_w = topk_w / (topk_w.sum(1, keepdims=True) + 1e-8)          # [T, k]

    tok_lists = []
    tok_weights = []
    for e in range(E):
        sel = (topk_idx == e)
        toks = np.where(sel.any(1))[0]
        w = (topk_w * sel)[toks].sum(1).astype(np.float32)
        tok_lists.append(toks)
        tok_weights.append(w)
    plan = tuple(len(t) for t in tok_lists)
    assert sum(plan) == RCOLS
    tok_concat = np.concatenate(tok_lists)                 # [4096]
    w_concat = np.concatenate(tok_weights)                 # [4096]

    # ---- x image [128, KH*NCOL], segment-major so each segment's load is
    # one fully contiguous DMA: seg block [p, k*n + j] = x[col oc+j, k*128+p]
    x_bf = x.astype(_bf)
    xT = np.empty((H, NCOL), _bf)
    xT[:, :RCOLS] = x_bf[tok_concat].T
    xT[:, RCOLS:] = x_bf.T
    xk = xT.reshape(KH, 128, NCOL)
    segs = _segments(plan)
    ximg = np.empty((128, KH * NCOL), _bf)
    for _, _, oc, n in segs:
        blk = xk[:, :, oc:oc + n].transpose(1, 0, 2).reshape(128, KH * n)
        ximg[:, KH * oc:KH * (oc + n)] = blk

    # ---- per-core weight images ------------------------------------------
    gT = [np.ascontiguousarray(np.asarray(routed_gate[e], np.float32).T)
          .astype(_bf) for e in range(E)]
    uT = [np.ascontiguousarray(np.asarray(routed_up[e], np.float32).T)
          .astype(_bf) for e in range(E)]
    dT = [np.ascontiguousarray(np.asarray(routed_down[e], np.float32).T)
          .astype(_bf) for e in range(E)]
    for s in range(NS):
        gT.append(np.ascontiguousarray(
            np.asarray(shared_gate[s], np.float32).T).astype(_bf))
        uT.append(np.ascontiguousarray(
            np.asarray(shared_up[s], np.float32).T).astype(_bf))
        dT.append(np.ascontiguousarray(
            np.asarray(shared_down[s], np.float32).T).astype(_bf))

    in_maps = []
    for c in range(N_CORES):
        wch = np.empty((N_EXP, 128, WROW), _bf)
        for e in range(N_EXP):
            wch[e, :, :CHUNK] = _chunk_gu(gT[e], c)
            wch[e, :, CHUNK:2 * CHUNK] = _chunk_gu(uT[e], c)
            wch[e, :, 2 * CHUNK:] = _chunk_d(dT[e], c)
        in_maps.append({"x": ximg, "wch": wch})

    # ---- build + run on 8 cores -----------------------------------------
    if plan not in _NC_CACHE:
        _NC_CACHE.clear()
        _NC_CACHE[plan] = build_nc(plan)
    nc = _NC_CACHE[plan]

    trace = bool(int(os.environ.get("MOE_TRACE", "0")))
    kw = {}
    if trace:
        _install_ntff_hook()
        kw = dict(trace=True, trace_cores=list(range(N_CORES)))
    res = run_bass_kernel_spmd(nc, in_maps, core_ids=list(range(N_CORES)), **kw)
    if trace:
        print(f"HW exec time: {res.exec_time_ns} ns")

    # ---- host combine: sum I-slice partials, weight, scatter -------------
    acc = np.zeros((128, KH * NCOL), np.float32)
    for c in range(N_CORES):
        acc += res.results[c]["y"].astype(np.float32)
    accT = np.empty((H, NCOL), np.float32)                 # [768, 6144]
    for _, _, oc, n in segs:
        blk = acc[:, KH * oc:KH * (oc + n)].reshape(128, KH, n)
        accT[:, oc:oc + n] = blk.transpose(1, 0, 2).reshape(H, n)
    out = np.zeros((T, H), np.float32)
    np.add.at(out, tok_concat, accT[:, :RCOLS].T * w_concat[:, None])
    out += accT[:, RCOLS:].T / NS
    return out.reshape(B, S, H)


# revision 40
# speedup vs baseline: 1.0357x; 1.0133x over previous
"""DeepSeek-MoE layer on 8 TRN2 NeuronCores.

Strategy (intermediate-dim parallel, host-side dispatch):
  - Router (x @ gate_w.T, softmax, top-2) computed on host — it *is* the
    sharding decision (~0.02% of total FLOPs).
  - Every core computes a 384-wide I-slice (1/8 of I=3072) of ALL 8
    routed experts over their routed tokens plus BOTH shared experts
    over all 2048 tokens.  Work per core is exactly 1/8 of the total
    MACs regardless of routing skew — no max-expert-load capacity
    penalty (expert-parallel pays max_e n_e on every core).
  - Cores emit partial down-projections (their I-slice's contribution,
    bf16); the host sums the 8 partials in fp32 and applies the combine
    weights / scatter-add.
  - All matmuls bf16 (fp32 PSUM accumulation).

Device kernel layout (driven by trace analysis):
  - Each dma_start costs ~0.65us of serialized HWDGE sequencer time, so
    transfers are merged: ONE weight DMA per expert (1.77 MB), ONE x
    DMA per col-tile segment, ONE merged output store per segment
    (expert 0 / segment 0 are split finer to shorten the critical path
    to the first matmul).
  - PSUM: 2 gate/up banks + 6 down banks = 8.  The six down-proj
    accumulations of k-slice j are emitted right after gate/up of
    slice j+1, so PE bubbles from the silu/mul round trip are filled
    and down banks accumulate across all h tiles with no half-drain.
  - 32 dummy N=256 matmuls on a memset tile run during the initial
    DMA wait so the PE_HAM clock gate is at 8/8 before real work
    (and so the idle-window monitor never re-throttles the PE clock).
  - Shared experts' down projections accumulate into the same PSUM
    banks (6-step) so the S0+S1 sum leaves the chip as one tensor.
"""
import os
import sys
import types

import numpy as np
import ml_dtypes

import concourse.bass as bass
import concourse.tile as tile
import concourse.mybir as mybir
from concourse import bacc
from concourse.bass_utils import run_bass_kernel_spmd

# ---- problem constants (DeepSeekMoE: B=2,S=1024,H=768,I=3072,E=8,NS=2,k=2) --
H = 768          # hidden
I = 3072         # intermediate
E = 8            # routed experts
NS = 2           # shared experts
TOP_K = 2
N_CORES = 8
KH = H // 128        # 6 k-tiles over H
ISL = I // N_CORES   # 384: per-core I-slice
MI = ISL // 128      # 3 mi-tiles per core per expert
T_TOT = 2048
RCOLS = T_TOT * TOP_K      # 4096 routed columns
SCOLS = T_TOT              # 2048 shared columns
NCOL = RCOLS + SCOLS       # 6144
CHUNK = KH * ISL           # 2304 elements per partition per weight matrix
WROW = 3 * CHUNK           # 6912: gate ++ up ++ down image per expert
N_EXP = E + NS             # 10 weight sets
WARM_MMS = 32

BF16 = mybir.dt.bfloat16
F32 = mybir.dt.float32
_bf = ml_dtypes.bfloat16


def _install_ntff_hook():
    """Provide antenv.axon_hooks (missing on this image) so trace=True works."""
    if "antenv.axon_hooks" in sys.modules:
        return
    try:
        from trn_agent_boot.trn_boot import _ntff_profile_via_ctypes
        hook = _ntff_profile_via_ctypes("/opt/axon/libaxon_pjrt.so")
    except Exception:
        hook = None
    mod = types.ModuleType("antenv.axon_hooks")
    mod.get_axon_ntff_profile_hook = lambda: hook
    sys.modules["antenv.axon_hooks"] = mod


def _col_tiles(n):
    """Split n columns into tiles <= 512."""
    if n == 0:
        return []
    nt = -(-n // 512)
    base = n // nt
    return [base + (1 if i < n % nt else 0) for i in range(nt)]


def _segments(plan):
    """Segment list: (kind, expert, col_offset, n_cols).  The shared range
    ends with a small 128-col segment so the final drain tail is short."""
    segs = []
    off = 0
    for e in range(E):
        for n in _col_tiles(plan[e]):
            segs.append(("r", e, off, n))
            off += n
    assert off == RCOLS
    for n in (512, 512, 512, 384, 128):
        segs.append(("s", 0, off, n))
        off += n
    assert off == NCOL
    return segs


def build_nc(plan):
    """Build the SPMD program.  plan: per-routed-expert token counts."""
    nc = bacc.Bacc(None, target_bir_lowering=False, enable_partition_id=False)
    X = nc.dram_tensor("x", [128, KH * NCOL], BF16, kind="ExternalInput")
    WCH = nc.dram_tensor("wch", [N_EXP, 128, WROW], BF16,
                         kind="ExternalInput")
    Y = nc.dram_tensor("y", [128, KH * NCOL], BF16, kind="ExternalOutput")

    segs = _segments(plan)
    nseg = len(segs)
    soff = [KH * s[2] for s in segs]  # segment-major element offsets

    with tile.TileContext(nc) as tc:
        with tc.tile_pool(name="wpool", bufs=4) as wpool, \
             tc.tile_pool(name="xpool", bufs=5) as xpool, \
             tc.tile_pool(name="hpool", bufs=10) as hpool, \
             tc.tile_pool(name="sgpool", bufs=3) as sgpool, \
             tc.tile_pool(name="ystage", bufs=6) as ystage, \
             tc.tile_pool(name="warmp", bufs=1) as warmp, \
             tc.tile_pool(name="gupool", bufs=2, space="PSUM") as gupool, \
             tc.tile_pool(name="ypool", bufs=6, space="PSUM") as ypool:

            # ---- HAM pre-warm: dummy matmuls on a memset tile ----------
            wsb = warmp.tile([128, 384], BF16, tag="warm", name="warmsrc")
            nc.vector.memset(wsb, 0.0)
            warm = gupool.tile([128, 512], F32, tag="gu", name="warmps")
            for _ in range(WARM_MMS):
                nc.tensor.matmul(warm[:, :256], wsb[:, :128], wsb[:, 128:384],
                                 start=True, stop=True)
            # preload the ACT COPY activation table (sel=1) off the critical
            # path — otherwise the first PSUM drain stalls scalar ~1.3us
            nc.scalar.copy(wsb[:, :8], wsb[:, 8:16])

            w_tiles = {}     # expert id (0..7 routed, 8/9 shared) -> tile
            x_tiles = {}     # segment index -> x tile

            def load_w(eid, split=False):
                if eid in w_tiles:
                    return
                wt = wpool.tile([128, WROW], BF16, tag="w")
                if split:
                    # finer pieces so the first matmuls wait on ~0.1MB
                    cuts = [0, ISL, CHUNK, 2 * CHUNK, 3 * CHUNK]
                else:
                    cuts = [0, WROW]
                for lo, hi in zip(cuts[:-1], cuts[1:]):
                    nc.sync.dma_start(out=wt[:, lo:hi],
                                      in_=WCH[eid, :, lo:hi])
                w_tiles[eid] = wt

            def issue_seg(si):
                kind, e, oc, n = segs[si]
                xt = xpool.tile([128, KH * 512], BF16, tag="x")
                so = soff[si]
                if si == 0:
                    # startup head: x0 whole, then gate/up/down chunks in
                    # consumption order — few DMAs (issue costs ~0.65us
                    # of ring-sequencer time each), warm matmuls bridge
                    nc.sync.dma_start(out=xt[:, :KH * n],
                                      in_=X[:, so:so + KH * n])
                    wt = wpool.tile([128, WROW], BF16, tag="w")
                    for lo, hi in ((0, CHUNK), (CHUNK, 2 * CHUNK),
                                   (2 * CHUNK, WROW)):
                        nc.sync.dma_start(out=wt[:, lo:hi],
                                          in_=WCH[e, :, lo:hi])
                    w_tiles[e] = wt
                    x_tiles[si] = xt
                    return
                nc.sync.dma_start(out=xt[:, :KH * n],
                                  in_=X[:, so:so + KH * n])
                x_tiles[si] = xt
                if kind == "r":
                    load_w(e)
                else:
                    load_w(E)
                    load_w(E + 1)

            seg_ctr = [0]

            def compute_seg(si):
                kind, e, oc, n = segs[si]
                xt = x_tiles.pop(si)
                eids = [e] if kind == "r" else [E, E + 1]
                nacc = len(eids) * MI
                ys = [ypool.tile([128, 512], F32, tag="y", name=f"y{t}")
                      for t in range(KH)]
                hs = {}

                def emit_down(gk):
                    eid_, j = eids[gk // MI], gk % MI
                    td = w_tiles[eid_]
                    for t in range(KH):
                        nc.tensor.matmul(
                            ys[t][:, :n],
                            td[:, 2 * CHUNK + j * H + t * 128:
                               2 * CHUNK + j * H + (t + 1) * 128],
                            hs[gk][:, :n],
                            start=(gk == 0), stop=(gk == nacc - 1))

                for ei, eid in enumerate(eids):
                    wt = w_tiles[eid]
                    for mi in range(MI):
                        gk = ei * MI + mi
                        g = gupool.tile([128, 512], F32, tag="gu")
                        for k in range(KH):
                            o = k * ISL + mi * 128
                            nc.tensor.matmul(
                                g[:, :n], wt[:, o:o + 128],
                                xt[:, k * n:k * n + n],
                                start=(k == 0), stop=(k == KH - 1))
                        u = gupool.tile([128, 512], F32, tag="gu")
                        for k in range(KH):
                            o = CHUNK + k * ISL + mi * 128
                            nc.tensor.matmul(
                                u[:, :n], wt[:, o:o + 128],
                                xt[:, k * n:k * n + n],
                                start=(k == 0), stop=(k == KH - 1))
                        sg = sgpool.tile([128, 512], F32, tag="sg")
                        nc.scalar.activation(sg[:, :n], g[:, :n],
                                             mybir.ActivationFunctionType.Silu)
                        h = hpool.tile([128, 512], BF16, tag="h")
                        nc.vector.tensor_mul(h[:, :n], sg[:, :n], u[:, :n])
                        hs[gk] = h
                        # down-proj lags two h tiles behind so its last
                        # groups never wait on the final silu/mul chain
                        if gk > 1:
                            emit_down(gk - 2)
                emit_down(nacc - 2)
                emit_down(nacc - 1)

                st = ystage.tile([128, KH * 512], BF16, tag="yst")
                for t in range(KH):
                    # alternate DVE/ACT so drains never serialize one queue
                    if t % 2 == 0:
                        nc.vector.tensor_copy(st[:, t * n:(t + 1) * n],
                                              ys[t][:, :n])
                    else:
                        nc.scalar.copy(st[:, t * n:(t + 1) * n],
                                       ys[t][:, :n])
                so = soff[si]
                nc.scalar.dma_start(out=Y[:, so:so + KH * n],
                                    in_=st[:, :KH * n])
                seg_ctr[0] += 1

            PF = 3  # segment prefetch depth
            for si in range(min(PF, nseg)):
                issue_seg(si)
            for si in range(nseg):
                if si + PF < nseg:
                    issue_seg(si + PF)
                compute_seg(si)
    nc.finalize()
    return nc


def _chunk_gu(wT, c):
    """[H, I] lhsT-layout weight -> this core's [128, 2304] gate/up image.
    chunk[p, k*384 + m] = wT[k*128 + p, c*384 + m]"""
    a = wT[:, c * ISL:(c + 1) * ISL].reshape(KH, 128, ISL)
    return np.ascontiguousarray(a.transpose(1, 0, 2)).reshape(128, CHUNK)


def _chunk_d(dT, c):
    """[I, H] lhsT-layout down weight -> this core's [128, 2304] image.
    chunk[p, j*768 + i] = dT[c*384 + j*128 + p, i]"""
    a = dT[c * ISL:(c + 1) * ISL, :].reshape(MI, 128, H)
    return np.ascontiguousarray(a.transpose(1, 0, 2)).reshape(128, CHUNK)


_NC_CACHE = {}


def kernel(hidden_states, gate_w, shared_gate, shared_up, shared_down,
           routed_gate, routed_up, routed_down):
    B, S, _ = hidden_states.shape
    T = B * S
    x = np.asarray(hidden_states, np.float32).reshape(T, H)

    # ---- host router (mirrors reference math; fp64 softmax for stability) --
    logits = x @ np.asarray(gate_w, np.float32).T                    # [T, E]
    lg = logits.astype(np.float64)
    sc = np.exp(lg - lg.max(1, keepdims=True))
    sc /= sc.sum(1, keepdims=True)
    topk_idx = np.argsort(-sc, axis=1, kind="stable")[:, :TOP_K]     # [T, k]
    topk_w = np.take_along_axis(sc, topk_idx, axis=1)
    topk_w = topk_w / (topk_w.sum(1, keepdims=True) + 1e-8)          # [T, k]

    tok_lists = []
    tok_weights = []
    for e in range(E):
        sel = (topk_idx == e)
        toks = np.where(sel.any(1))[0]
        w = (topk_w * sel)[toks].sum(1).astype(np.float32)
        tok_lists.append(toks)
        tok_weights.append(w)
    plan = tuple(len(t) for t in tok_lists)
    assert sum(plan) == RCOLS
    tok_concat = np.concatenate(tok_lists)                 # [4096]
    w_concat = np.concatenate(tok_weights)                 # [4096]

    # ---- x image [128, KH*NCOL], segment-major so each segment's load is
    # one fully contiguous DMA: seg block [p, k*n + j] = x[col oc+j, k*128+p]
    x_bf = x.astype(_bf)
    xT = np.empty((H, NCOL), _bf)
    xT[:, :RCOLS] = x_bf[tok_concat].T
    xT[:, RCOLS:] = x_bf.T
    xk = xT.reshape(KH, 128, NCOL)
    segs = _segments(plan)
    ximg = np.empty((128, KH * NCOL), _bf)
    for _, _, oc, n in segs:
        blk = xk[:, :, oc:oc + n].transpose(1, 0, 2).reshape(128, KH * n)
        ximg[:, KH * oc:KH * (oc + n)] = blk

    # ---- per-core weight images ------------------------------------------
    gT = [np.ascontiguousarray(np.asarray(routed_gate[e], np.float32).T)
          .astype(_bf) for e in range(E)]
    uT = [np.ascontiguousarray(np.asarray(routed_up[e], np.float32).T)
          .astype(_bf) for e in range(E)]
    dT = [np.ascontiguousarray(np.asarray(routed_down[e], np.float32).T)
          .astype(_bf) for e in range(E)]
    for s in range(NS):
        gT.append(np.ascontiguousarray(
            np.asarray(shared_gate[s], np.float32).T).astype(_bf))
        uT.append(np.ascontiguousarray(
            np.asarray(shared_up[s], np.float32).T).astype(_bf))
        dT.append(np.ascontiguousarray(
            np.asarray(shared_down[s], np.float32).T).astype(_bf))

    in_maps = []
    for c in range(N_CORES):
        wch = np.empty((N_EXP, 128, WROW), _bf)
        for e in range(N_EXP):
            wch[e, :, :CHUNK] = _chunk_gu(gT[e], c)
            wch[e, :, CHUNK:2 * CHUNK] = _chunk_gu(uT[e], c)
            wch[e, :, 2 * CHUNK:] = _chunk_d(dT[e], c)
        in_maps.append({"x": ximg, "wch": wch})

    # ---- build + run on 8 cores -----------------------------------------
    if plan not in _NC_CACHE:
        _NC_CACHE.clear()
        _NC_CACHE[plan] = build_nc(plan)
    nc = _NC_CACHE[plan]

    trace = bool(int(os.environ.get("MOE_TRACE", "0")))
    kw = {}
    if trace:
        _install_ntff_hook()
        kw = dict(trace=True, trace_cores=list(range(N_CORES)))
    res = run_bass_kernel_spmd(nc, in_maps, core_ids=list(range(N_CORES)), **kw)
    if trace:
        print(f"HW exec time: {res.exec_time_ns} ns")

    # ---- host combine: sum I-slice partials, weight, scatter -------------
    acc = np.zeros((128, KH * NCOL), np.float32)
    for c in range(N_CORES):
        acc += res.results[c]["y"].astype(np.float32)
    accT = np.empty((H, NCOL), np.float32)                 # [768, 6144]
    for _, _, oc, n in segs:
        blk = acc[:, KH * oc:KH * (oc + n)].reshape(128, KH, n)
        accT[:, oc:oc + n] = blk.transpose(1, 0, 2).reshape(H, n)
    out = np.zeros((T, H), np.float32)
    np.add.at(out, tok_concat, accT[:, :RCOLS].T * w_concat[:, None])
    out += accT[:, RCOLS:].T / NS
    return out.reshape(B, S, H)
